# revision 11
# baseline (speedup 1.0000x reference)
"""Trainium2 Bass kernel for nn_Condensation: 10 sequential masked-blur
composites over [16,3,768,768], data-parallel across 8 NeuronCores (2 images,
6 image-channels per core).

Structure (vs the padded-box baseline):
  - tight input/output boxes per drop (support eps 1e-4 / composite eps 3e-4),
    global row offset chosen to minimize 128-row blocks touched per drop
  - separable blur as two banded-matmul passes (image/intermediate stationary,
    band conv matrices streaming), pass-A output restricted to output rows
  - composite out' = out + m*(B - out) with two build-time variants per
    drop-group: (A) oq = out - om on a vector engine off the critical path,
    (B) -I @ state matmul injected into the pass-B PSUM accumulation
  - all drop parameters DMA'd up front; image loaded/stored as per-block
    column intervals keyed to their first/last-touching drop (host supplies
    everything outside the output-box union from the exact f32 input)
  - elementwise+eviction ops assigned to vector/gpsimd/scalar by a greedy
    cost-model balancer at build time
"""
import numpy as np
import ml_dtypes

NUM_DROPS = 10
MIN_R, MAX_R = 60.0, 80.0
BETA = 1.8
BLUR_RADII = [11.3535, 17.9381, 5.7966, 10.8586, 5.5301, 15.9075, 12.3225, 13.4871, 6.6639, 9.5413]


def _ksize(r):
    k = int(2 * r) + 1
    return k + 1 if k % 2 == 0 else k


KSIZES = [_ksize(r) for r in BLUR_RADII]
H = W = 768
P = 128
B_TOTAL, C = 16, 3
N_CORES = 8
B_LOC = B_TOTAL // N_CORES
NG = B_LOC * C // 2                  # 3 groups of 2 image-channels
EPS_IN = 1e-4
EPS_OUT = 3e-4

_bf16 = ml_dtypes.bfloat16


def _conv_matrix(sigma, ksize, n=768):
    half = (ksize - 1) * 0.5
    xs = np.linspace(-half, half, ksize)
    pdf = np.exp(-0.5 * (xs / np.float64(sigma)) ** 2)
    k1 = (pdf / pdf.sum()).astype(np.float32).astype(np.float64)
    pad = ksize // 2
    Kmat = np.zeros((n, n), dtype=np.float64)
    idx = np.arange(n)[:, None] + np.arange(ksize)[None, :] - pad
    idx = np.abs(idx)
    idx = np.where(idx >= n, 2 * n - 2 - idx, idx)
    np.add.at(Kmat, (np.repeat(np.arange(n), ksize), idx.ravel()), np.tile(k1, n))
    return Kmat.astype(np.float32)


class _Drop:
    pass


class _Meta:
    pass


def _drop_meta(positions, radius):
    pos = np.clip(np.asarray(positions, np.float32), -1.0, 1.0)
    rad = np.clip(np.asarray(radius, np.float32), MIN_R, MAX_R)
    wv = np.arange(W, dtype=np.float32)[None, :]
    s_in = float(np.sqrt((np.log(1.0 / EPS_IN)) ** (1.0 / BETA)))
    s_out = float(np.sqrt((np.log(1.0 / EPS_OUT)) ** (1.0 / BETA)))
    drops = []
    for j in range(NUM_DROPS):
        d = _Drop()
        d.j = j
        x0 = (pos[j, 0] + 1.0) / 2.0 * W
        y0 = (pos[j, 1] + 1.0) / 2.0 * H
        d.x0, d.y0 = float(x0), float(y0)
        wr = float(rad[j])
        hr = wr * 0.8
        d.wr, d.hr = wr, hr
        ks = KSIZES[j]
        p = ks // 2
        d.p = p
        # output box, even-aligned rows+cols
        ho0 = max(0, int(np.floor(y0 - s_out * hr))) & ~1
        ho1 = min(H, (int(np.ceil(y0 + s_out * hr)) + 2) & ~1)
        wo0 = max(0, int(np.floor(x0 - s_out * wr))) & ~1
        wo1 = min(W, (int(np.ceil(x0 + s_out * wr)) + 2) & ~1)
        # input box = (output box +- p) intersect support box
        hs0 = max(0, int(np.floor(y0 - s_in * hr)))
        hs1 = min(H, int(np.ceil(y0 + s_in * hr)) + 1)
        ws0 = max(0, int(np.floor(x0 - s_in * wr)))
        ws1 = min(W, int(np.ceil(x0 + s_in * wr)) + 1)
        hi0 = max(max(0, ho0 - p), hs0) & ~1
        hi1 = min(min(H, (ho1 + p + 1) & ~1), (hs1 + 1) & ~1)
        wi0 = (max(max(0, wo0 - p), ws0)) & ~1
        wi1 = min(min(W, (wo1 + p + 1) & ~1), (ws1 + 1) & ~1)
        d.ho0, d.ho1, d.wo0, d.wo1 = ho0, ho1, wo0, wo1
        d.hi0, d.hi1, d.wi0, d.wi1 = hi0, hi1, wi0, wi1
        d.Wr = wo1 - wo0
        d.Wwi = wi1 - wi0
        d.WBn = (d.Wwi + P - 1) // P
        drops.append(d)

    # global row offset: minimize blocks touched (weighted by op volume)
    min_hi0 = min(d.hi0 for d in drops)
    max_hi1 = max(d.hi1 for d in drops)
    best = None
    for row0 in range(max(0, min_hi0 - 127), min_hi0 + 1, 2):
        nb = (max_hi1 - row0 + P - 1) // P
        if row0 + nb * P > H:
            continue
        cost = 0
        for d in drops:
            kbn = (d.hi1 - row0 + P - 1) // P - (d.hi0 - row0) // P
            obn = (d.ho1 - row0 + P - 1) // P - (d.ho0 - row0) // P
            cost += 2 * kbn * d.Wwi + 6 * obn * d.Wr
        key = (nb, cost)
        if best is None or key < best[0]:
            best = (key, row0)
    row0 = best[1]
    meta = _Meta()
    meta.row0 = row0
    meta.NB = (max_hi1 - row0 + P - 1) // P

    wv = np.arange(W, dtype=np.float32)[None, :]
    for d in drops:
        d.KB0 = (d.hi0 - row0) // P
        d.KBn = (d.hi1 - row0 + P - 1) // P - d.KB0
        d.OB0 = (d.ho0 - row0) // P
        d.OBn = (d.ho1 - row0 + P - 1) // P - d.OB0
        d.OGRID = d.OBn * P
        assert d.OB0 >= d.KB0 and d.OB0 + d.OBn <= d.KB0 + d.KBn

        # mask on input block grid (absolute rows row0 + (KB0+k)*P + p_)
        rows = row0 + d.KB0 * P + np.arange(d.KBn * P)
        dd = (rows[:, None].astype(np.float32) - d.y0) ** 2 / d.hr ** 2 + \
             (wv[:, d.wi0:d.wi1] - d.x0) ** 2 / d.wr ** 2
        m = np.clip(np.exp(-(dd.astype(np.float32) ** np.float32(BETA)) + np.float32(1e-10)), 0.0, 1.0)
        mz = np.zeros_like(m)
        r0, r1 = d.hi0 - (row0 + d.KB0 * P), d.hi1 - (row0 + d.KB0 * P)
        mz[r0:r1] = m[r0:r1]
        d.m_np = np.ascontiguousarray(
            mz.reshape(d.KBn, P, d.Wwi).transpose(1, 0, 2)).astype(_bf16)

        MT = _conv_matrix(BLUR_RADII[d.j], KSIZES[d.j])
        # pass A: MvT_dev[p_, k, q] = Mv[out_row, in_row]^T
        d.aA, d.bA = [], []
        for k in range(d.KBn):
            blo = row0 + (d.KB0 + k) * P
            lo = max(d.ho0, blo - d.p)
            hi = min(d.ho1, blo + P + d.p)
            d.aA.append(lo - (row0 + d.OB0 * P))
            d.bA.append(hi - (row0 + d.OB0 * P))
        bandA = max(b - a for a, b in zip(d.aA, d.bA))
        bandA = (bandA + 1) & ~1
        d.bandA = bandA
        MvT = np.zeros((P, d.KBn, bandA), np.float32)
        for k in range(d.KBn):
            in_rows = row0 + (d.KB0 + k) * P + np.arange(P)
            out_rows = row0 + d.OB0 * P + d.aA[k] + np.arange(d.bA[k] - d.aA[k])
            MvT[:, k, :d.bA[k] - d.aA[k]] = MT[np.ix_(out_rows, in_rows)].T
        d.MvT_np = MvT.astype(_bf16)

        # pass B: MhT_dev[p_, wc, q] = Mh[out_col, in_col]^T
        d.aB, d.bB = [], []
        for wc in range(d.WBn):
            lo = max(d.wo0, d.wi0 + wc * P - d.p)
            hi = min(d.wo1, d.wi0 + (wc + 1) * P + d.p)
            hi = max(hi, lo)
            d.aB.append(lo - d.wo0)
            d.bB.append(hi - d.wo0)
        bandB = max(b - a for a, b in zip(d.aB, d.bB))
        bandB = (bandB + 1) & ~1
        d.bandB = bandB
        MhT = np.zeros((P, d.WBn, bandB), np.float32)
        for wc in range(d.WBn):
            cw = min(P, d.Wwi - wc * P)
            in_cols = d.wi0 + wc * P + np.arange(cw)
            out_cols = d.wo0 + d.aB[wc] + np.arange(d.bB[wc] - d.aB[wc])
            MhT[:cw, wc, :d.bB[wc] - d.aB[wc]] = MT[np.ix_(out_cols, in_cols)].T
        d.MhT_np = MhT.astype(_bf16)
    meta.drops = drops

    # ---- per-block column-interval load/store plans
    # atomic intervals from box boundaries; first/last toucher per (block, ival)
    NB = meta.NB
    meta.loads = []    # (nb, w0, w1, first_drop)
    meta.stores = []   # (nb, w0, w1, last_drop)
    for nb in range(NB):
        # drops whose input (load) / output (store) box touches this block
        lts = [(d.wi0, d.wi1, d.j) for d in drops if d.KB0 <= nb < d.KB0 + d.KBn]
        sts = [(d.wo0, d.wo1, d.j) for d in drops if d.OB0 <= nb < d.OB0 + d.OBn]
        for touch, out in ((lts, meta.loads), (sts, meta.stores)):
            cuts = sorted({w for t in touch for w in t[:2]})
            ivals = []
            for a, b in zip(cuts[:-1], cuts[1:]):
                js = [j for (w0, w1, j) in touch if w0 <= a and b <= w1]
                if js:
                    ivals.append((a, b, min(js) if out is meta.loads else max(js)))
            # merge adjacent with same key drop
            merged = []
            for a, b, j in ivals:
                if merged and merged[-1][1] == a and merged[-1][2] == j:
                    merged[-1] = (merged[-1][0], b, j)
                else:
                    merged.append((a, b, j))
            # coalesce narrow intervals (tiny DMA descriptors are inefficient):
            # loads may merge earlier (min), stores later (max)
            red = min if out is meta.loads else max
            changed = True
            while changed and len(merged) > 1:
                changed = False
                for i, (a, b, j) in enumerate(merged):
                    if b - a < 192:
                        if i + 1 < len(merged) and merged[i + 1][0] == b:
                            merged[i + 1] = (a, merged[i + 1][1], red(j, merged[i + 1][2]))
                            del merged[i]
                        elif i > 0 and merged[i - 1][1] == a:
                            merged[i - 1] = (merged[i - 1][0], b, red(merged[i - 1][2], j))
                            del merged[i]
                        else:
                            continue
                        changed = True
                        break
            out.extend((nb, a, b, j) for a, b, j in merged)
    meta.loads.sort(key=lambda t: (t[3], t[0], t[1]))
    meta.stores.sort(key=lambda t: (t[3], t[0], t[1]))
    return meta


# ---------------------------------------------------------------------------
# build-time engine balancer
class _Balance:
    def __init__(self):
        self.t = {'V': 0.0, 'G': 0.0, 'S': 0.0, 'T': 0.0}

    @staticmethod
    def cost(eng, fd, psum=False):
        # calibrated on HW traces: DVE TT ~1 elem/ns (1x + pipe drain),
        # gpsimd TT ~0.45 elem/ns, scalar ACTIVATE copy ~3.3 elem/ns
        if eng == 'V':
            return (150 if psum else 150) + fd * 1.0
        if eng == 'G':
            return 300 + fd * 1.1
        if eng == 'S':
            return 50 + fd / 4.8
        raise KeyError(eng)

    def pick(self, nc, fd, psum=False, copy=False):
        if psum:
            cands = ['V', 'S']          # gpsimd cannot access PSUM
        elif copy:
            cands = ['V', 'G', 'S']
        else:
            cands = ['V', 'G']
        best = min(cands, key=lambda e: self.t[e] + self.cost(e, fd, psum))
        self.t[best] += self.cost(best, fd, psum)
        eng = {'V': nc.vector, 'G': nc.gpsimd, 'S': nc.scalar}[best]
        return eng, best


def _build_program(meta):
    from contextlib import ExitStack
    from concourse import bacc, tile, mybir

    f32 = mybir.dt.float32
    bf16 = mybir.dt.bfloat16
    drops = meta.drops
    NB = meta.NB
    row0 = meta.row0

    nc = bacc.Bacc("TRN2", target_bir_lowering=False, debug=False,
                   num_devices=N_CORES)
    img_d = nc.declare_dram_parameter("img", [B_LOC, C, H, W], bf16, False)
    out_d = nc.declare_dram_parameter("out", [B_LOC, C, NB * P, W], bf16, True)
    negI_d = nc.declare_dram_parameter("negI", [P, P], bf16, False)
    dparams = []
    for d in drops:
        m_d = nc.declare_dram_parameter(f"m{d.j}", [P, d.KBn, d.Wwi], bf16, False)
        mv_d = nc.declare_dram_parameter(f"mv{d.j}", [P, d.KBn, d.bandA], bf16, False)
        mh_d = nc.declare_dram_parameter(f"mh{d.j}", [P, d.WBn, d.bandB], bf16, False)
        dparams.append((m_d, mv_d, mh_d))

    bal = _Balance()

    with tile.TileContext(nc) as tc, ExitStack() as ctx:
        statep = ctx.enter_context(tc.tile_pool(name="state", bufs=1))
        state = [statep.tile([P, 2, NB, W], bf16, name=f"st{g}", tag=f"st{g}")
                 for g in range(NG)]
        parp = ctx.enter_context(tc.tile_pool(name="params", bufs=1))
        negI_t = parp.tile([P, P], bf16, tag="negI")
        ptiles = []
        for d, (m_d, mv_d, mh_d) in zip(drops, dparams):
            m_t = parp.tile([P, d.KBn, d.Wwi], bf16, tag=f"m{d.j}")
            mv_t = parp.tile([P, d.KBn, d.bandA], bf16, tag=f"mv{d.j}")
            mh_t = parp.tile([P, d.WBn, d.bandB], bf16, tag=f"mh{d.j}")
            ptiles.append((m_t, mv_t, mh_t))
        omp = ctx.enter_context(tc.tile_pool(name="om", bufs=5))
        vtp = ctx.enter_context(tc.tile_pool(name="vt", bufs=1))
        bsp = ctx.enter_context(tc.tile_pool(name="bs", bufs=3))
        t2p = ctx.enter_context(tc.tile_pool(name="t2", bufs=3))
        oqp = ctx.enter_context(tc.tile_pool(name="oq", bufs=3))
        ppa = ctx.enter_context(tc.tile_pool(name="psa", bufs=2, space="PSUM"))
        ppb = ctx.enter_context(tc.tile_pool(name="psb", bufs=2, space="PSUM"))

        # image viewed as [p, (b c), nb, w] over the present rows
        img_r = img_d.ap()[:, :, row0:row0 + NB * P, :].rearrange(
            "b c (n p) w -> p (b c) n w", p=P)
        out_r = out_d.ap().rearrange("b c (n p) w -> p (b c) n w", p=P)

        # ---- DMA schedule, head-optimized:
        #   sync queue:   g0/g1 image loads (block first-touch order), then all stores
        #   scalar queue: params (drop order) interleaved with g2 image loads
        first_touch = {}
        for (nb, a, b, jfirst) in meta.loads:
            first_touch[nb] = min(first_touch.get(nb, 99), jfirst)
        blocks_seq = sorted(first_touch, key=lambda nb: first_touch[nb])
        for nb in blocks_seq:
            for g in (0, 1):
                nc.sync.dma_start(out=state[g][:, :, nb, :],
                                  in_=img_r[:, 2 * g:2 * g + 2, nb, :])
        g2_loaded = set()

        def load_g2(upto_drop):
            for nb in blocks_seq:
                if nb not in g2_loaded and first_touch[nb] <= upto_drop:
                    g2_loaded.add(nb)
                    nc.scalar.dma_start(out=state[2][:, :, nb, :],
                                        in_=img_r[:, 4:6, nb, :])

        nc.scalar.dma_start(out=negI_t[:], in_=negI_d.ap()[:])
        for di, (d, (m_d, mv_d, mh_d), (m_t, mv_t, mh_t)) in enumerate(
                zip(drops, dparams, ptiles)):
            nc.scalar.dma_start(out=m_t[:], in_=m_d.ap()[:])
            nc.scalar.dma_start(out=mv_t[:], in_=mv_d.ap()[:])
            nc.scalar.dma_start(out=mh_t[:], in_=mh_d.ap()[:])
            load_g2(di)
        load_g2(99)

        # vt pool buffers hold garbage cols outside written ranges on first use;
        # zero them once so pass-B lhsT never streams NaN bit patterns
        vts_all = [vtp.tile([P, 2, 512], bf16, name=f"vtz{i}", tag=f"vtz{i}", bufs=1)
                   for i in range(12)]
        for t in vts_all:
            nc.gpsimd.memset(t[:], 0.0)
        vt_idx = [0]

        def vt_tile():
            t = vts_all[vt_idx[0] % 12]
            vt_idx[0] += 1
            return t

        stores_by_drop = {}
        for (nb, a, b, jlast) in meta.stores:
            stores_by_drop.setdefault(jlast, []).append((nb, a, b))

        def emit_om(d, g):
            m_t = ptiles[d.j][0]
            om = omp.tile([P, 2, d.KBn, d.Wwi], bf16, tag="om")
            for j in range(2):
                eng, _ = bal.pick(nc, d.KBn * d.Wwi)
                eng.tensor_mul(om[:, j], m_t[:],
                               state[g][:, j, d.KB0:d.KB0 + d.KBn, d.wi0:d.wi1])
            return om

        def emit_passA(d, g, om):
            mv_t = ptiles[d.j][1]
            vts = []
            ho0r = d.ho0 - (row0 + d.OB0 * P)
            ho1r = d.ho1 - (row0 + d.OB0 * P)
            for wc in range(d.WBn):
                cw = min(P, d.Wwi - wc * P)
                psa = ppa.tile([P, 2, 512], f32, tag="psa")
                for j in range(2):
                    for k in range(d.KBn):
                        a, b = d.aA[k], d.bA[k]
                        nc.tensor.matmul(
                            psa[0:cw, j, a:b],
                            lhsT=om[:, j, k, wc * P:wc * P + cw],
                            rhs=mv_t[:, k, 0:b - a],
                            start=(k == 0), stop=(k == d.KBn - 1))
                vt = vt_tile()
                fd = 2 * (ho1r - ho0r)
                eng, _ = bal.pick(nc, fd, psum=True, copy=True)
                if eng is nc.scalar:
                    eng.copy(vt[:, :, ho0r:ho1r], psa[:, :, ho0r:ho1r])
                else:
                    eng.tensor_copy(vt[:, :, ho0r:ho1r], psa[:, :, ho0r:ho1r])
                vts.append((vt, cw))
            return vts

        def emit_passB_composite(d, g, om, vts, variant):
            mh_t = ptiles[d.j][2]
            m_t = ptiles[d.j][0]
            Bs = bsp.tile([P, 2, d.OBn, d.Wr], bf16, tag="bs")
            if variant == 'A':
                oq = oqp.tile([P, 2, d.OBn, d.Wr], bf16, tag="oq")
                eng, _ = bal.pick(nc, 2 * d.OBn * d.Wr)
                eng.tensor_sub(
                    oq[:],
                    state[g][:, :, d.OB0:d.OB0 + d.OBn, d.wo0:d.wo1],
                    om[:, :, d.OB0 - d.KB0:d.OB0 - d.KB0 + d.OBn,
                       d.wo0 - d.wi0:d.wo0 - d.wi0 + d.Wr])  # om has j dim
            for hb in range(d.OBn):
                psb = ppb.tile([P, 2, 512], f32, tag="psb")
                for j in range(2):
                    for wc in range(d.WBn):
                        a, b = d.aB[wc], d.bB[wc]
                        if b <= a:
                            continue
                        vt, cw = vts[wc]
                        nc.tensor.matmul(
                            psb[0:P, j, a:b],
                            lhsT=vt[0:cw, j, hb * P:(hb + 1) * P],
                            rhs=mh_t[0:cw, wc, 0:b - a],
                            start=(wc == 0), stop=(wc == d.WBn - 1 and variant == 'A'))
                    if variant == 'B':
                        nc.tensor.matmul(
                            psb[:, j, 0:d.Wr],
                            lhsT=negI_t[:],
                            rhs=state[g][:, j, d.OB0 + hb, d.wo0:d.wo1],
                            start=False, stop=True)
                fd = 2 * d.Wr
                eng, _ = bal.pick(nc, fd, psum=True, copy=True)
                if eng is nc.scalar:
                    eng.copy(Bs[:, :, hb, :], psb[:, :, 0:d.Wr])
                else:
                    eng.tensor_copy(Bs[:, :, hb, :], psb[:, :, 0:d.Wr])
            # t2 = m * Bs  (mask sliced to output window)
            t2 = t2p.tile([P, 2, d.OBn, d.Wr], bf16, tag="t2")
            mo = m_t[:, d.OB0 - d.KB0:d.OB0 - d.KB0 + d.OBn,
                     d.wo0 - d.wi0:d.wo0 - d.wi0 + d.Wr]
            for j in range(2):
                eng, _ = bal.pick(nc, d.OBn * d.Wr)
                eng.tensor_mul(t2[:, j], mo, Bs[:, j])
            osl = state[g][:, :, d.OB0:d.OB0 + d.OBn, d.wo0:d.wo1]
            eng, _ = bal.pick(nc, 2 * d.OBn * d.Wr)
            if variant == 'A':
                eng.tensor_add(osl, oq[:], t2[:])
            else:
                eng.tensor_add(osl, osl, t2[:])

        def pick_variant(d):
            # variant B moves the oq subtract from V/G to tensor
            inj = 2 * d.OBn * (d.Wr / 2.4 + 60)
            oqc = _Balance.cost('V', 2 * d.OBn * d.Wr)
            if bal.t['T'] + inj < min(bal.t['V'], bal.t['G']) + oqc:
                return 'B'
            return 'A'

        def tensor_cost(d, variant):
            c = 0.0
            for wc in range(d.WBn):
                cw = min(P, d.Wwi - wc * P)
                for k in range(d.KBn):
                    c += max(cw / 2.4, (d.bA[k] - d.aA[k]) / 2.4 + 10) * 2
            for hb in range(d.OBn):
                for wc in range(d.WBn):
                    c += max(128 / 2.4, (d.bB[wc] - d.aB[wc]) / 2.4 + 10) * 2
                if variant == 'B':
                    c += 2 * (d.Wr / 2.4 + 55)
            return c

        # ---- main software-pipelined loop
        oms = {g: emit_om(drops[0], g) for g in range(NG)}
        for di, d in enumerate(drops):
            vts_g = {}
            for g in range(NG):
                vts_g[g] = emit_passA(d, g, oms[g])
            next_oms = {}
            for g in range(NG):
                variant = pick_variant(d)
                bal.t['T'] += tensor_cost(d, variant)
                emit_passB_composite(d, g, oms[g], vts_g[g], variant)
                if di + 1 < len(drops):
                    next_oms[g] = emit_om(drops[di + 1], g)
                # stores whose last toucher is this drop
                if di in stores_by_drop:
                    for (nb, a, b) in stores_by_drop[di]:
                        nc.sync.dma_start(out=out_r[:, 2 * g:2 * g + 2, nb, a:b],
                                          in_=state[g][:, :, nb, a:b])
            oms = next_oms

    nc.compile()
    return nc


_CACHE = {}


def _get_program(positions, radius):
    key = (np.asarray(positions, np.float32).tobytes(),
           np.asarray(radius, np.float32).tobytes())
    if key not in _CACHE:
        meta = _drop_meta(positions, radius)
        _CACHE[key] = (_build_program(meta), meta)
    return _CACHE[key]


def kernel(img, positions, radius, _want_trace=False, **_kw):
    from concourse.bass_utils import run_bass_kernel_spmd
    img = np.asarray(img, np.float32)
    assert img.shape == (B_TOTAL, C, H, W)
    nc, meta = _get_program(positions, radius)

    shards = np.ascontiguousarray(img.astype(_bf16)).reshape(
        N_CORES, B_LOC, C, H, W)
    base = {"negI": (-np.eye(P, dtype=np.float32)).astype(_bf16)}
    for d in meta.drops:
        base[f"m{d.j}"] = d.m_np
        base[f"mv{d.j}"] = d.MvT_np
        base[f"mh{d.j}"] = d.MhT_np
    in_maps = [dict(base, img=shards[i]) for i in range(N_CORES)]
    res = run_bass_kernel_spmd(nc, in_maps, core_ids=list(range(N_CORES)),
                               trace=_want_trace)
    out = img.copy().reshape(N_CORES, B_LOC, C, H, W)
    row0 = meta.row0
    for i in range(N_CORES):
        dev = np.asarray(res.results[i]["out"]).astype(np.float32)
        for d in meta.drops:
            out[i, :, :, d.ho0:d.ho1, d.wo0:d.wo1] = \
                dev[:, :, d.ho0 - row0:d.ho1 - row0, d.wo0:d.wo1]
    out = out.reshape(B_TOTAL, C, H, W)
    if _want_trace:
        return out, res
    return out


def simulate(img, positions, radius):
    """Numpy simulation of the exact device computation (for validation)."""
    meta = _drop_meta(positions, radius)
    row0 = meta.row0
    img = np.asarray(img, np.float32)
    Bc_, Cc = img.shape[0], img.shape[1]

    def Q(x):
        return np.asarray(x, np.float32).astype(_bf16).astype(np.float32)

    x = np.array(Q(img).reshape(Bc_ * Cc, H, W))
    st = x[:, row0:row0 + meta.NB * P, :].copy()
    for d in meta.drops:
        mi = d.m_np.astype(np.float32).transpose(1, 0, 2).reshape(d.KBn * P, d.Wwi)
        MvT = d.MvT_np.astype(np.float32)
        MhT = d.MhT_np.astype(np.float32)
        kb0, ob0 = d.KB0 * P, d.OB0 * P
        sl = st[:, kb0:kb0 + d.KBn * P, d.wi0:d.wi1]
        om = Q(sl * mi[None])
        vt = np.zeros((st.shape[0], d.Wwi, d.OGRID), np.float32)
        ho0r = d.ho0 - (row0 + ob0)
        ho1r = d.ho1 - (row0 + ob0)
        for k in range(d.KBn):
            a, b = d.aA[k], d.bA[k]
            vt[:, :, a:b] += np.einsum(
                'bhw,hq->bwq', om[:, k * P:(k + 1) * P, :], MvT[:, k, :b - a])
        vte = np.zeros_like(vt)
        vte[:, :, ho0r:ho1r] = Q(vt[:, :, ho0r:ho1r])
        Bc = np.zeros((st.shape[0], d.OGRID, d.Wr), np.float32)
        for wc in range(d.WBn):
            a, b = d.aB[wc], d.bB[wc]
            cw = min(P, d.Wwi - wc * P)
            Bc[:, :, a:b] += np.einsum(
                'bwq,wo->bqo', vte[:, wc * P:wc * P + cw, :], MhT[:cw, wc, :b - a])
        osl = st[:, ob0:ob0 + d.OGRID, d.wo0:d.wo1]
        mo = mi[(d.OB0 - d.KB0) * P:(d.OB0 - d.KB0) * P + d.OGRID,
                d.wo0 - d.wi0:d.wo0 - d.wi0 + d.Wr]
        omo = om[:, (d.OB0 - d.KB0) * P:(d.OB0 - d.KB0) * P + d.OGRID,
                 d.wo0 - d.wi0:d.wo0 - d.wi0 + d.Wr]
        oq = Q(osl - omo)
        t2 = Q(mo[None] * Q(Bc))
        st[:, ob0:ob0 + d.OGRID, d.wo0:d.wo1] = Q(oq + t2)
    for d in meta.drops:
        x[:, d.ho0:d.ho1, d.wo0:d.wo1] = \
            st[:, d.ho0 - row0:d.ho1 - row0, d.wo0:d.wo1]
    # exact f32 outside output boxes
    xf = np.array(img.reshape(Bc_ * Cc, H, W))
    for d in meta.drops:
        xf[:, d.ho0:d.ho1, d.wo0:d.wo1] = x[:, d.ho0:d.ho1, d.wo0:d.wo1]
    return xf.reshape(Bc_, Cc, H, W)


if __name__ == '__main__':
    img = np.load('/root/problem/img_input.npy')
    positions = np.load('/root/problem/pos_input.npy')
    radius = np.load('/root/problem/rad_input.npy')
    expected = np.load('/root/problem/expected.npy')
    out = simulate(img, positions, radius)
    err = np.abs(out - expected)
    scale = np.abs(expected).max()
    print(f"simulate: max_abs_err={err.max():.4e} rel={err.max()/scale:.4e}")


# revision 12
# speedup vs baseline: 1.2734x; 1.2734x over previous
"""Trainium2 Bass kernel for nn_Condensation: 10 sequential masked-blur
composites over [16,3,768,768], data-parallel across 8 NeuronCores (2 images,
6 image-channels per core).

Structure (vs the padded-box baseline):
  - tight input/output boxes per drop (support eps 1e-4 / composite eps 3e-4),
    global row offset chosen to minimize 128-row blocks touched per drop
  - separable blur as two banded-matmul passes (image/intermediate stationary,
    band conv matrices streaming), pass-A output restricted to output rows
  - composite out' = out + m*(B - out) with two build-time variants per
    drop-group: (A) oq = out - om on a vector engine off the critical path,
    (B) -I @ state matmul injected into the pass-B PSUM accumulation
  - all drop parameters DMA'd up front; image loaded/stored as per-block
    column intervals keyed to their first/last-touching drop (host supplies
    everything outside the output-box union from the exact f32 input)
  - elementwise+eviction ops assigned to vector/gpsimd/scalar by a greedy
    cost-model balancer at build time
"""
import numpy as np
import ml_dtypes

NUM_DROPS = 10
MIN_R, MAX_R = 60.0, 80.0
BETA = 1.8
BLUR_RADII = [11.3535, 17.9381, 5.7966, 10.8586, 5.5301, 15.9075, 12.3225, 13.4871, 6.6639, 9.5413]


def _ksize(r):
    k = int(2 * r) + 1
    return k + 1 if k % 2 == 0 else k


KSIZES = [_ksize(r) for r in BLUR_RADII]
H = W = 768
P = 128
B_TOTAL, C = 16, 3
N_CORES = 8
B_LOC = B_TOTAL // N_CORES
NG = B_LOC * C // 2                  # 3 groups of 2 image-channels
EPS_IN = 1e-4
EPS_OUT = 3e-4

_bf16 = ml_dtypes.bfloat16


def _conv_matrix(sigma, ksize, n=768):
    half = (ksize - 1) * 0.5
    xs = np.linspace(-half, half, ksize)
    pdf = np.exp(-0.5 * (xs / np.float64(sigma)) ** 2)
    k1 = (pdf / pdf.sum()).astype(np.float32).astype(np.float64)
    pad = ksize // 2
    Kmat = np.zeros((n, n), dtype=np.float64)
    idx = np.arange(n)[:, None] + np.arange(ksize)[None, :] - pad
    idx = np.abs(idx)
    idx = np.where(idx >= n, 2 * n - 2 - idx, idx)
    np.add.at(Kmat, (np.repeat(np.arange(n), ksize), idx.ravel()), np.tile(k1, n))
    return Kmat.astype(np.float32)


class _Drop:
    pass


class _Meta:
    pass


def _drop_meta(positions, radius):
    pos = np.clip(np.asarray(positions, np.float32), -1.0, 1.0)
    rad = np.clip(np.asarray(radius, np.float32), MIN_R, MAX_R)
    wv = np.arange(W, dtype=np.float32)[None, :]
    s_in = float(np.sqrt((np.log(1.0 / EPS_IN)) ** (1.0 / BETA)))
    s_out = float(np.sqrt((np.log(1.0 / EPS_OUT)) ** (1.0 / BETA)))
    drops = []
    for j in range(NUM_DROPS):
        d = _Drop()
        d.j = j
        x0 = (pos[j, 0] + 1.0) / 2.0 * W
        y0 = (pos[j, 1] + 1.0) / 2.0 * H
        d.x0, d.y0 = float(x0), float(y0)
        wr = float(rad[j])
        hr = wr * 0.8
        d.wr, d.hr = wr, hr
        ks = KSIZES[j]
        p = ks // 2
        d.p = p
        # output box, even-aligned rows+cols
        ho0 = max(0, int(np.floor(y0 - s_out * hr))) & ~1
        ho1 = min(H, (int(np.ceil(y0 + s_out * hr)) + 2) & ~1)
        wo0 = max(0, int(np.floor(x0 - s_out * wr))) & ~1
        wo1 = min(W, (int(np.ceil(x0 + s_out * wr)) + 2) & ~1)
        # input box = (output box +- p) intersect support box
        hs0 = max(0, int(np.floor(y0 - s_in * hr)))
        hs1 = min(H, int(np.ceil(y0 + s_in * hr)) + 1)
        ws0 = max(0, int(np.floor(x0 - s_in * wr)))
        ws1 = min(W, int(np.ceil(x0 + s_in * wr)) + 1)
        hi0 = max(max(0, ho0 - p), hs0) & ~1
        hi1 = min(min(H, (ho1 + p + 1) & ~1), (hs1 + 1) & ~1)
        wi0 = (max(max(0, wo0 - p), ws0)) & ~1
        wi1 = min(min(W, (wo1 + p + 1) & ~1), (ws1 + 1) & ~1)
        d.ho0, d.ho1, d.wo0, d.wo1 = ho0, ho1, wo0, wo1
        d.hi0, d.hi1, d.wi0, d.wi1 = hi0, hi1, wi0, wi1
        d.Wr = wo1 - wo0
        d.Wwi = wi1 - wi0
        d.WBn = (d.Wwi + P - 1) // P
        drops.append(d)

    # global row offset: minimize blocks touched (weighted by op volume)
    min_hi0 = min(d.hi0 for d in drops)
    max_hi1 = max(d.hi1 for d in drops)
    best = None
    for row0 in range(max(0, min_hi0 - 127), min_hi0 + 1, 2):
        nb = (max_hi1 - row0 + P - 1) // P
        if row0 + nb * P > H:
            continue
        cost = 0
        for d in drops:
            kbn = (d.hi1 - row0 + P - 1) // P - (d.hi0 - row0) // P
            obn = (d.ho1 - row0 + P - 1) // P - (d.ho0 - row0) // P
            cost += 2 * kbn * d.Wwi + 6 * obn * d.Wr
        key = (nb, cost)
        if best is None or key < best[0]:
            best = (key, row0)
    row0 = best[1]
    meta = _Meta()
    meta.row0 = row0
    meta.NB = (max_hi1 - row0 + P - 1) // P

    wv = np.arange(W, dtype=np.float32)[None, :]
    for d in drops:
        d.KB0 = (d.hi0 - row0) // P
        d.KBn = (d.hi1 - row0 + P - 1) // P - d.KB0
        d.OB0 = (d.ho0 - row0) // P
        d.OBn = (d.ho1 - row0 + P - 1) // P - d.OB0
        d.OGRID = d.OBn * P
        assert d.OB0 >= d.KB0 and d.OB0 + d.OBn <= d.KB0 + d.KBn

        # mask on input block grid (absolute rows row0 + (KB0+k)*P + p_)
        rows = row0 + d.KB0 * P + np.arange(d.KBn * P)
        dd = (rows[:, None].astype(np.float32) - d.y0) ** 2 / d.hr ** 2 + \
             (wv[:, d.wi0:d.wi1] - d.x0) ** 2 / d.wr ** 2
        m = np.clip(np.exp(-(dd.astype(np.float32) ** np.float32(BETA)) + np.float32(1e-10)), 0.0, 1.0)
        mz = np.zeros_like(m)
        r0, r1 = d.hi0 - (row0 + d.KB0 * P), d.hi1 - (row0 + d.KB0 * P)
        mz[r0:r1] = m[r0:r1]
        d.m_np = np.ascontiguousarray(
            mz.reshape(d.KBn, P, d.Wwi).transpose(1, 0, 2)).astype(_bf16)

        MT = _conv_matrix(BLUR_RADII[d.j], KSIZES[d.j])
        # pass A: MvT_dev[p_, k, q] = Mv[out_row, in_row]^T
        d.aA, d.bA = [], []
        for k in range(d.KBn):
            blo = row0 + (d.KB0 + k) * P
            lo = max(d.ho0, blo - d.p)
            hi = min(d.ho1, blo + P + d.p)
            d.aA.append(lo - (row0 + d.OB0 * P))
            d.bA.append(hi - (row0 + d.OB0 * P))
        bandA = max(b - a for a, b in zip(d.aA, d.bA))
        bandA = (bandA + 1) & ~1
        d.bandA = bandA
        MvT = np.zeros((P, d.KBn, bandA), np.float32)
        for k in range(d.KBn):
            in_rows = row0 + (d.KB0 + k) * P + np.arange(P)
            out_rows = row0 + d.OB0 * P + d.aA[k] + np.arange(d.bA[k] - d.aA[k])
            MvT[:, k, :d.bA[k] - d.aA[k]] = MT[np.ix_(out_rows, in_rows)].T
        d.MvT_np = MvT.astype(_bf16)

        # pass B: MhT_dev[p_, wc, q] = Mh[out_col, in_col]^T
        d.aB, d.bB = [], []
        for wc in range(d.WBn):
            lo = max(d.wo0, d.wi0 + wc * P - d.p)
            hi = min(d.wo1, d.wi0 + (wc + 1) * P + d.p)
            hi = max(hi, lo)
            d.aB.append(lo - d.wo0)
            d.bB.append(hi - d.wo0)
        bandB = max(b - a for a, b in zip(d.aB, d.bB))
        bandB = (bandB + 1) & ~1
        d.bandB = bandB
        MhT = np.zeros((P, d.WBn, bandB), np.float32)
        for wc in range(d.WBn):
            cw = min(P, d.Wwi - wc * P)
            in_cols = d.wi0 + wc * P + np.arange(cw)
            out_cols = d.wo0 + d.aB[wc] + np.arange(d.bB[wc] - d.aB[wc])
            MhT[:cw, wc, :d.bB[wc] - d.aB[wc]] = MT[np.ix_(out_cols, in_cols)].T
        d.MhT_np = MhT.astype(_bf16)
    meta.drops = drops

    # ---- per-block column-interval load/store plans
    # atomic intervals from box boundaries; first/last toucher per (block, ival)
    NB = meta.NB
    meta.loads = []    # (nb, w0, w1, first_drop)
    meta.stores = []   # (nb, w0, w1, last_drop)
    for nb in range(NB):
        # drops whose input (load) / output (store) box touches this block
        lts = [(d.wi0, d.wi1, d.j) for d in drops if d.KB0 <= nb < d.KB0 + d.KBn]
        sts = [(d.wo0, d.wo1, d.j) for d in drops if d.OB0 <= nb < d.OB0 + d.OBn]
        for touch, out in ((lts, meta.loads), (sts, meta.stores)):
            cuts = sorted({w for t in touch for w in t[:2]})
            ivals = []
            for a, b in zip(cuts[:-1], cuts[1:]):
                js = [j for (w0, w1, j) in touch if w0 <= a and b <= w1]
                if js:
                    ivals.append((a, b, min(js) if out is meta.loads else max(js)))
            # merge adjacent with same key drop
            merged = []
            for a, b, j in ivals:
                if merged and merged[-1][1] == a and merged[-1][2] == j:
                    merged[-1] = (merged[-1][0], b, j)
                else:
                    merged.append((a, b, j))
            # coalesce narrow intervals (tiny DMA descriptors are inefficient):
            # loads may merge earlier (min), stores later (max)
            red = min if out is meta.loads else max
            changed = True
            while changed and len(merged) > 1:
                changed = False
                for i, (a, b, j) in enumerate(merged):
                    if b - a < 192:
                        if i + 1 < len(merged) and merged[i + 1][0] == b:
                            merged[i + 1] = (a, merged[i + 1][1], red(j, merged[i + 1][2]))
                            del merged[i]
                        elif i > 0 and merged[i - 1][1] == a:
                            merged[i - 1] = (merged[i - 1][0], b, red(merged[i - 1][2], j))
                            del merged[i]
                        else:
                            continue
                        changed = True
                        break
            out.extend((nb, a, b, j) for a, b, j in merged)
    meta.loads.sort(key=lambda t: (t[3], t[0], t[1]))
    meta.stores.sort(key=lambda t: (t[3], t[0], t[1]))
    return meta


# ---------------------------------------------------------------------------
# build-time engine balancer
class _Balance:
    def __init__(self):
        self.t = {'V': 0.0, 'G': 0.0, 'S': 0.0, 'T': 0.0}

    @staticmethod
    def cost(eng, fd, psum=False):
        # calibrated on HW traces: DVE TT ~1 elem/ns (1x + pipe drain),
        # gpsimd TT ~0.45 elem/ns, scalar ACTIVATE copy ~3.3 elem/ns
        if eng == 'V':
            return (150 if psum else 150) + fd * 1.0
        if eng == 'G':
            return 300 + fd * 2.2
        if eng == 'S':
            return 50 + fd / 4.8
        raise KeyError(eng)

    def pick(self, nc, fd, psum=False, copy=False):
        if psum:
            cands = ['V', 'S']          # gpsimd cannot access PSUM
        elif copy:
            cands = ['V', 'G', 'S']
        else:
            cands = ['V', 'G']
        best = min(cands, key=lambda e: self.t[e] + self.cost(e, fd, psum))
        self.t[best] += self.cost(best, fd, psum)
        eng = {'V': nc.vector, 'G': nc.gpsimd, 'S': nc.scalar}[best]
        return eng, best


def _build_program(meta):
    from contextlib import ExitStack
    from concourse import bacc, tile, mybir

    f32 = mybir.dt.float32
    bf16 = mybir.dt.bfloat16
    drops = meta.drops
    NB = meta.NB
    row0 = meta.row0

    nc = bacc.Bacc("TRN2", target_bir_lowering=False, debug=False,
                   num_devices=N_CORES)
    img_d = nc.declare_dram_parameter("img", [B_LOC, C, H, W], bf16, False)
    out_d = nc.declare_dram_parameter("out", [B_LOC, C, NB * P, W], bf16, True)
    negI_d = nc.declare_dram_parameter("negI", [P, P], bf16, False)
    dparams = []
    for d in drops:
        m_d = nc.declare_dram_parameter(f"m{d.j}", [P, d.KBn, d.Wwi], bf16, False)
        mv_d = nc.declare_dram_parameter(f"mv{d.j}", [P, d.KBn, d.bandA], bf16, False)
        mh_d = nc.declare_dram_parameter(f"mh{d.j}", [P, d.WBn, d.bandB], bf16, False)
        dparams.append((m_d, mv_d, mh_d))

    bal = _Balance()

    with tile.TileContext(nc) as tc, ExitStack() as ctx:
        statep = ctx.enter_context(tc.tile_pool(name="state", bufs=1))
        state = [statep.tile([P, 2, NB, W], bf16, name=f"st{g}", tag=f"st{g}")
                 for g in range(NG)]
        parp = ctx.enter_context(tc.tile_pool(name="params", bufs=1))
        negI_t = parp.tile([P, P], bf16, tag="negI")
        ptiles = []
        for d, (m_d, mv_d, mh_d) in zip(drops, dparams):
            m_t = parp.tile([P, d.KBn, d.Wwi], bf16, tag=f"m{d.j}")
            mv_t = parp.tile([P, d.KBn, d.bandA], bf16, tag=f"mv{d.j}")
            mh_t = parp.tile([P, d.WBn, d.bandB], bf16, tag=f"mh{d.j}")
            ptiles.append((m_t, mv_t, mh_t))
        omp = ctx.enter_context(tc.tile_pool(name="om", bufs=5))
        vtp = ctx.enter_context(tc.tile_pool(name="vt", bufs=1))
        bsp = ctx.enter_context(tc.tile_pool(name="bs", bufs=3))
        t2p = ctx.enter_context(tc.tile_pool(name="t2", bufs=3))
        oqp = ctx.enter_context(tc.tile_pool(name="oq", bufs=3))
        ppa = ctx.enter_context(tc.tile_pool(name="psa", bufs=2, space="PSUM"))
        ppb = ctx.enter_context(tc.tile_pool(name="psb", bufs=2, space="PSUM"))

        # image viewed as [p, (b c), nb, w] over the present rows
        img_r = img_d.ap()[:, :, row0:row0 + NB * P, :].rearrange(
            "b c (n p) w -> p (b c) n w", p=P)
        out_r = out_d.ap().rearrange("b c (n p) w -> p (b c) n w", p=P)

        # ---- DMA schedule, head-optimized:
        #   sync queue:   g0/g1 image loads (block first-touch order), then all stores
        #   scalar queue: params (drop order) interleaved with g2 image loads
        first_touch = {}
        for (nb, a, b, jfirst) in meta.loads:
            first_touch[nb] = min(first_touch.get(nb, 99), jfirst)
        blocks_seq = sorted(first_touch, key=lambda nb: first_touch[nb])
        for nb in blocks_seq:
            for g in (0, 1):
                nc.sync.dma_start(out=state[g][:, :, nb, :],
                                  in_=img_r[:, 2 * g:2 * g + 2, nb, :])
        g2_loaded = set()

        def load_g2(upto_drop):
            for nb in blocks_seq:
                if nb not in g2_loaded and first_touch[nb] <= upto_drop:
                    g2_loaded.add(nb)
                    nc.scalar.dma_start(out=state[2][:, :, nb, :],
                                        in_=img_r[:, 4:6, nb, :])

        nc.scalar.dma_start(out=negI_t[:], in_=negI_d.ap()[:])
        for di, (d, (m_d, mv_d, mh_d), (m_t, mv_t, mh_t)) in enumerate(
                zip(drops, dparams, ptiles)):
            nc.scalar.dma_start(out=m_t[:], in_=m_d.ap()[:])
            nc.scalar.dma_start(out=mv_t[:], in_=mv_d.ap()[:])
            nc.scalar.dma_start(out=mh_t[:], in_=mh_d.ap()[:])
            load_g2(di)
        load_g2(99)

        # vt pool buffers hold garbage cols outside written ranges on first use;
        # zero them once so pass-B lhsT never streams NaN bit patterns
        vts_all = [vtp.tile([P, 2, 512], bf16, name=f"vtz{i}", tag=f"vtz{i}", bufs=1)
                   for i in range(12)]
        for t in vts_all:
            nc.gpsimd.memset(t[:], 0.0)
        vt_idx = [0]

        def vt_tile():
            t = vts_all[vt_idx[0] % 12]
            vt_idx[0] += 1
            return t

        stores_by_drop = {}
        for (nb, a, b, jlast) in meta.stores:
            stores_by_drop.setdefault(jlast, []).append((nb, a, b))

        def emit_om(d, g):
            m_t = ptiles[d.j][0]
            om = omp.tile([P, 2, d.KBn, d.Wwi], bf16, tag="om")
            for j in range(2):
                eng, _ = bal.pick(nc, d.KBn * d.Wwi)
                eng.tensor_mul(om[:, j], m_t[:],
                               state[g][:, j, d.KB0:d.KB0 + d.KBn, d.wi0:d.wi1])
            return om

        def emit_passA(d, g, om):
            mv_t = ptiles[d.j][1]
            vts = []
            ho0r = d.ho0 - (row0 + d.OB0 * P)
            ho1r = d.ho1 - (row0 + d.OB0 * P)
            for wc in range(d.WBn):
                cw = min(P, d.Wwi - wc * P)
                psa = ppa.tile([P, 2, 512], f32, tag="psa")
                for j in range(2):
                    for k in range(d.KBn):
                        a, b = d.aA[k], d.bA[k]
                        nc.tensor.matmul(
                            psa[0:cw, j, a:b],
                            lhsT=om[:, j, k, wc * P:wc * P + cw],
                            rhs=mv_t[:, k, 0:b - a],
                            start=(k == 0), stop=(k == d.KBn - 1))
                vt = vt_tile()
                fd = 2 * (ho1r - ho0r)
                eng, _ = bal.pick(nc, fd, psum=True, copy=True)
                if eng is nc.scalar:
                    eng.copy(vt[:, :, ho0r:ho1r], psa[:, :, ho0r:ho1r])
                else:
                    eng.tensor_copy(vt[:, :, ho0r:ho1r], psa[:, :, ho0r:ho1r])
                vts.append((vt, cw))
            return vts

        def emit_passB_composite(d, g, om, vts, variant):
            mh_t = ptiles[d.j][2]
            m_t = ptiles[d.j][0]
            Bs = bsp.tile([P, 2, d.OBn, d.Wr], bf16, tag="bs")
            if variant == 'A':
                oq = oqp.tile([P, 2, d.OBn, d.Wr], bf16, tag="oq")
                eng, _ = bal.pick(nc, 2 * d.OBn * d.Wr)
                eng.tensor_sub(
                    oq[:],
                    state[g][:, :, d.OB0:d.OB0 + d.OBn, d.wo0:d.wo1],
                    om[:, :, d.OB0 - d.KB0:d.OB0 - d.KB0 + d.OBn,
                       d.wo0 - d.wi0:d.wo0 - d.wi0 + d.Wr])  # om has j dim
            for hb in range(d.OBn):
                psb = ppb.tile([P, 2, 512], f32, tag="psb")
                for j in range(2):
                    for wc in range(d.WBn):
                        a, b = d.aB[wc], d.bB[wc]
                        if b <= a:
                            continue
                        vt, cw = vts[wc]
                        nc.tensor.matmul(
                            psb[0:P, j, a:b],
                            lhsT=vt[0:cw, j, hb * P:(hb + 1) * P],
                            rhs=mh_t[0:cw, wc, 0:b - a],
                            start=(wc == 0), stop=(wc == d.WBn - 1 and variant == 'A'))
                    if variant == 'B':
                        nc.tensor.matmul(
                            psb[:, j, 0:d.Wr],
                            lhsT=negI_t[:],
                            rhs=state[g][:, j, d.OB0 + hb, d.wo0:d.wo1],
                            start=False, stop=True)
                fd = 2 * d.Wr
                eng, _ = bal.pick(nc, fd, psum=True, copy=True)
                if eng is nc.scalar:
                    eng.copy(Bs[:, :, hb, :], psb[:, :, 0:d.Wr])
                else:
                    eng.tensor_copy(Bs[:, :, hb, :], psb[:, :, 0:d.Wr])
            # t2 = m * Bs  (mask sliced to output window)
            t2 = t2p.tile([P, 2, d.OBn, d.Wr], bf16, tag="t2")
            mo = m_t[:, d.OB0 - d.KB0:d.OB0 - d.KB0 + d.OBn,
                     d.wo0 - d.wi0:d.wo0 - d.wi0 + d.Wr]
            for j in range(2):
                eng, _ = bal.pick(nc, d.OBn * d.Wr)
                eng.tensor_mul(t2[:, j], mo, Bs[:, j])
            osl = state[g][:, :, d.OB0:d.OB0 + d.OBn, d.wo0:d.wo1]
            eng, _ = bal.pick(nc, 2 * d.OBn * d.Wr)
            if variant == 'A':
                eng.tensor_add(osl, oq[:], t2[:])
            else:
                eng.tensor_add(osl, osl, t2[:])

        def pick_variant(d):
            # variant B moves the oq subtract from V/G to tensor
            inj = 2 * d.OBn * (d.Wr / 2.4 + 60)
            oqc = _Balance.cost('V', 2 * d.OBn * d.Wr)
            if bal.t['T'] + inj < min(bal.t['V'], bal.t['G']) + oqc:
                return 'B'
            return 'A'

        def tensor_cost(d, variant):
            c = 0.0
            for wc in range(d.WBn):
                cw = min(P, d.Wwi - wc * P)
                for k in range(d.KBn):
                    c += max(cw / 2.4, (d.bA[k] - d.aA[k]) / 2.4 + 10) * 2
            for hb in range(d.OBn):
                for wc in range(d.WBn):
                    c += max(128 / 2.4, (d.bB[wc] - d.aB[wc]) / 2.4 + 10) * 2
                if variant == 'B':
                    c += 2 * (d.Wr / 2.4 + 55)
            return c

        # ---- main software-pipelined loop
        oms = {g: emit_om(drops[0], g) for g in range(NG)}
        for di, d in enumerate(drops):
            vts_g = {}
            for g in range(NG):
                vts_g[g] = emit_passA(d, g, oms[g])
            next_oms = {}
            for g in range(NG):
                variant = pick_variant(d)
                bal.t['T'] += tensor_cost(d, variant)
                emit_passB_composite(d, g, oms[g], vts_g[g], variant)
                if di + 1 < len(drops):
                    next_oms[g] = emit_om(drops[di + 1], g)
                # stores whose last toucher is this drop
                if di in stores_by_drop:
                    for (nb, a, b) in stores_by_drop[di]:
                        nc.sync.dma_start(out=out_r[:, 2 * g:2 * g + 2, nb, a:b],
                                          in_=state[g][:, :, nb, a:b])
            oms = next_oms

    nc.compile()
    return nc


_CACHE = {}


def _get_program(positions, radius):
    key = (np.asarray(positions, np.float32).tobytes(),
           np.asarray(radius, np.float32).tobytes())
    if key not in _CACHE:
        meta = _drop_meta(positions, radius)
        _CACHE[key] = (_build_program(meta), meta)
    return _CACHE[key]


def kernel(img, positions, radius, _want_trace=False, **_kw):
    from concourse.bass_utils import run_bass_kernel_spmd
    img = np.asarray(img, np.float32)
    assert img.shape == (B_TOTAL, C, H, W)
    nc, meta = _get_program(positions, radius)

    shards = np.ascontiguousarray(img.astype(_bf16)).reshape(
        N_CORES, B_LOC, C, H, W)
    base = {"negI": (-np.eye(P, dtype=np.float32)).astype(_bf16)}
    for d in meta.drops:
        base[f"m{d.j}"] = d.m_np
        base[f"mv{d.j}"] = d.MvT_np
        base[f"mh{d.j}"] = d.MhT_np
    in_maps = [dict(base, img=shards[i]) for i in range(N_CORES)]
    res = run_bass_kernel_spmd(nc, in_maps, core_ids=list(range(N_CORES)),
                               trace=_want_trace)
    out = img.copy().reshape(N_CORES, B_LOC, C, H, W)
    row0 = meta.row0
    for i in range(N_CORES):
        dev = np.asarray(res.results[i]["out"]).astype(np.float32)
        for d in meta.drops:
            out[i, :, :, d.ho0:d.ho1, d.wo0:d.wo1] = \
                dev[:, :, d.ho0 - row0:d.ho1 - row0, d.wo0:d.wo1]
    out = out.reshape(B_TOTAL, C, H, W)
    if _want_trace:
        return out, res
    return out


def simulate(img, positions, radius):
    """Numpy simulation of the exact device computation (for validation)."""
    meta = _drop_meta(positions, radius)
    row0 = meta.row0
    img = np.asarray(img, np.float32)
    Bc_, Cc = img.shape[0], img.shape[1]

    def Q(x):
        return np.asarray(x, np.float32).astype(_bf16).astype(np.float32)

    x = np.array(Q(img).reshape(Bc_ * Cc, H, W))
    st = x[:, row0:row0 + meta.NB * P, :].copy()
    for d in meta.drops:
        mi = d.m_np.astype(np.float32).transpose(1, 0, 2).reshape(d.KBn * P, d.Wwi)
        MvT = d.MvT_np.astype(np.float32)
        MhT = d.MhT_np.astype(np.float32)
        kb0, ob0 = d.KB0 * P, d.OB0 * P
        sl = st[:, kb0:kb0 + d.KBn * P, d.wi0:d.wi1]
        om = Q(sl * mi[None])
        vt = np.zeros((st.shape[0], d.Wwi, d.OGRID), np.float32)
        ho0r = d.ho0 - (row0 + ob0)
        ho1r = d.ho1 - (row0 + ob0)
        for k in range(d.KBn):
            a, b = d.aA[k], d.bA[k]
            vt[:, :, a:b] += np.einsum(
                'bhw,hq->bwq', om[:, k * P:(k + 1) * P, :], MvT[:, k, :b - a])
        vte = np.zeros_like(vt)
        vte[:, :, ho0r:ho1r] = Q(vt[:, :, ho0r:ho1r])
        Bc = np.zeros((st.shape[0], d.OGRID, d.Wr), np.float32)
        for wc in range(d.WBn):
            a, b = d.aB[wc], d.bB[wc]
            cw = min(P, d.Wwi - wc * P)
            Bc[:, :, a:b] += np.einsum(
                'bwq,wo->bqo', vte[:, wc * P:wc * P + cw, :], MhT[:cw, wc, :b - a])
        osl = st[:, ob0:ob0 + d.OGRID, d.wo0:d.wo1]
        mo = mi[(d.OB0 - d.KB0) * P:(d.OB0 - d.KB0) * P + d.OGRID,
                d.wo0 - d.wi0:d.wo0 - d.wi0 + d.Wr]
        omo = om[:, (d.OB0 - d.KB0) * P:(d.OB0 - d.KB0) * P + d.OGRID,
                 d.wo0 - d.wi0:d.wo0 - d.wi0 + d.Wr]
        oq = Q(osl - omo)
        t2 = Q(mo[None] * Q(Bc))
        st[:, ob0:ob0 + d.OGRID, d.wo0:d.wo1] = Q(oq + t2)
    for d in meta.drops:
        x[:, d.ho0:d.ho1, d.wo0:d.wo1] = \
            st[:, d.ho0 - row0:d.ho1 - row0, d.wo0:d.wo1]
    # exact f32 outside output boxes
    xf = np.array(img.reshape(Bc_ * Cc, H, W))
    for d in meta.drops:
        xf[:, d.ho0:d.ho1, d.wo0:d.wo1] = x[:, d.ho0:d.ho1, d.wo0:d.wo1]
    return xf.reshape(Bc_, Cc, H, W)


if __name__ == '__main__':
    img = np.load('/root/problem/img_input.npy')
    positions = np.load('/root/problem/pos_input.npy')
    radius = np.load('/root/problem/rad_input.npy')
    expected = np.load('/root/problem/expected.npy')
    out = simulate(img, positions, radius)
    err = np.abs(out - expected)
    scale = np.abs(expected).max()
    print(f"simulate: max_abs_err={err.max():.4e} rel={err.max()/scale:.4e}")
